# revision 48
# baseline (speedup 1.0000x reference)
"""GAT layer kernel for 8 Trainium2 NeuronCores (raw bass, explicit semaphores).

Math: the reference computes
    h_prime = node_feats @ w                      [N, D]
    s_src   = h_prime @ w_a @ a[:2]               [N]
    s_dst   = h_prime @ w_a @ a[2:]               [N]
    e[i,j]  = s_src[i] + s_dst[j], masked by Ahat, row-softmax, @ h_prime, relu.

Because e[i,j] separates into row + column terms, s_src[i] cancels in the
row-wise softmax.  With g[j] = exp(s_dst[j] - max_j s_dst[j]):
    attn[i,j] = Ahat[i,j] * g[j] / sum_j' Ahat[i,j'] * g[j']
    out       = relu( (Ahat @ [g*h_prime | g])[:, :D] / (Ahat @ g) )
i.e. the whole layer collapses to one big [N,N] x [N,D+1] matmul against the
adjacency matrix plus tiny preprocessing.  That matmul is the memory-bound
part (Ahat is 400 MB); everything else is noise.

Sharding: output rows are sharded across the 8 cores (slab of N/8 rows each).
The PE contracts along the partition dim, so each core receives its Ahat
row-slab pre-transposed (j on partitions) and cast to bf16 (exact for 0/1
adjacency values, and halves HBM traffic).  Both dims are zero-padded
(padding is inert: zero adjacency contributes nothing) so that
  - every stationary matmul tile is exactly 128 columns (enables FWL), and
  - the host pre-chunks the slab so each partition reads one contiguous
    10 KB run per block DMA (full descriptor rate).
h_prime/s_dst preprocessing runs redundantly on every core in fp32 (s_dst
feeds exp so it needs full precision; node_feats/w are tiny).

Raw bass (not Tile): this container's walrus rejects instructions carrying
more than ~2 sync waits (and >1 on self-loading fp32 matmuls), which Tile's
auto-generated kernel-tail drain always violates.  With explicit semaphores
every wait is its own instruction, which walrus accepts.
"""

import sys
from contextlib import ExitStack

sys.path.insert(0, "/opt/trn_rl_repo")

import ml_dtypes
import numpy as np

import concourse.bass as bass
from concourse import mybir
from concourse.bass_utils import run_bass_kernel_spmd

N = 10000
FIN = 256
D = 128
DEXT = D + 1          # h_prime columns + the g column (for the softmax denom)
NCORES = 8
SLAB = N // NCORES    # real output rows per core
JC = 128              # j-chunk (contraction tile)
RT = 128              # output row tile
C4 = 4                # j-chunks per A-tile DMA block
SB2 = 2               # DMA blocks per psum accumulation group
NP = 10240            # N padded to C4*JC*20
IP = 1280             # slab padded to RT*10
NBUF = 8              # A-tile buffer depth

F32 = mybir.dt.float32
BF16 = mybir.dt.bfloat16
FP8 = mybir.dt.float8e4
F32R = mybir.dt.float32r
AF = mybir.ActivationFunctionType
AX = mybir.AxisListType


def _build_program(np_=NP, ip=IP, nbuf=NBUF):
    nj = np_ // JC            # j-chunks
    npair = nj // 2           # preprocessing chunk pairs
    nr = ip // RT             # output row tiles per core
    nblk = nj // C4           # A-tile DMA blocks
    nsb = nblk // SB2         # psum accumulation superblocks
    cs = C4 * SB2             # chunks per accumulation group
    nps = 4                   # rotating psum banks for output accumulation
    bw = C4 * ip              # SBUF columns per A-tile block
    assert np_ % (SB2 * C4 * JC) == 0 and ip % RT == 0

    nc = bass.Bass("TRN2", target_bir_lowering=False, debug=False, num_devices=NCORES)

    # a_h: block-chunked slab layout [nblk*128, C4*ip]; row b*128+p holds, for
    # each chunk c of block b, the (padded) A^T row j = (b*C4+c)*128 + p.
    a_h = nc.declare_dram_parameter("a_h", [nblk * JC, bw], FP8, isOutput=False)
    nf_t = nc.declare_dram_parameter("nf_t", [FIN, np_], F32, isOutput=False)
    w_ext = nc.declare_dram_parameter("w_ext", [FIN, DEXT], F32, isOutput=False)
    out = nc.declare_dram_parameter("out", [ip, D], F32, isOutput=True)

    ctx = ExitStack()
    with ctx:
        sb = lambda name, shape, dt: ctx.enter_context(nc.sbuf_tensor(name, shape, dt))
        ps = lambda name, shape: ctx.enter_context(nc.psum_tensor(name, shape, F32))
        sem = lambda name: ctx.enter_context(nc.semaphore(name))

        nft_sb = sb("nft_sb", [128, 2 * np_], F32)
        wext_sb = sb("wext_sb", [128, 2 * DEXT], F32)
        hg_sb = sb("hg_sb", [128, nj * DEXT], BF16)
        sdst = sb("sdst", [128, nj], F32)
        gall = sb("gall", [128, nj], F32)
        mcol = sb("mcol", [128, 1], F32)
        gmax = sb("gmax", [128, 1], F32)
        negm = sb("negm", [128, 1], F32)
        negones = sb("negones", [128, 128], F32)
        zeros = sb("zeros", [128, 1], F32)
        ident = sb("ident", [128, 128], F32)
        rec = sb("rec", [128, nr], F32)
        zcl = sb("zcl", [128, nr], F32)
        res = sb("res", [128, nr * D], F32)
        racc = sb("racc", [128, nr * DEXT], F32)
        att = [sb(f"att{i}", [128, bw], FP8) for i in range(nbuf)]

        pband = [ps("pband0", [128, 2 * DEXT]), ps("pband1", [128, 2 * DEXT])]
        misc_ps = ps("misc_ps", [128, 128])
        outp = [ps(f"outp{p}", [128, DEXT]) for p in range(nps)]

        # strided column-views (one column per chunk / per pair)
        hg_gcol = hg_sb[:, :].rearrange("p (k e) -> p k e", e=DEXT)[:, :, D]

        pre_dma = sem("pre_dma")
        nft_q = [sem(f"nft_q{i}") for i in range(4)]
        # One semaphore per A-tile buffer slot: a shared counter would be
        # unsound (16 SDMA engines inc independently, so value 16 can be a mix
        # of two DMAs).  Slot reuse is serialized by the dve_add WAR gate.
        in_dma_s = [sem(f"in_dma{i}") for i in range(nbuf)]
        out_dma = sem("out_dma")
        pe_pre = sem("pe_pre")
        act_hp = sem("act_hp")    # one inc per preproc pair drained (ACT)
        dve_pre = sem("dve_pre")  # one inc per preproc pair s_dst extracted
        pe_misc = sem("pe_misc")
        act_misc = sem("act_misc")
        dve_sem = sem("dve_sem")
        act_hg = sem("act_hg")
        pe_grp = sem("pe_grp")    # one inc per output matmul
        dve_add = sem("dve_add")  # one inc per psum -> racc accumulate
        act_out = sem("act_out")
        gp_sem = sem("gp_sem")

        with nc.Block() as block:

            @block.gpsimd
            def _(gpsimd):
                # GpSimd is 8 Q7 cores; same-engine ops race without a sem.
                nc.gpsimd.memset(ident[:, :], 0.0).then_inc(gp_sem, 1)
                gpsimd.wait_ge(gp_sem, 1)
                nc.gpsimd.affine_select(
                    out=ident[:, :],
                    in_=ident[:, :],
                    compare_op=mybir.AluOpType.not_equal,
                    fill=1.0,
                    base=0,
                    pattern=[[-1, 128]],
                    channel_multiplier=1,
                ).then_inc(gp_sem, 1)

            @block.sync
            def _(sync):
                qw = np_ // 4  # nft quarter width (j columns)

                def nft_quarter(qi):
                    # both f-halves of j-quarter qi in one DMA
                    sync.dma_start(
                        out=nft_sb[:, :]
                        .rearrange("p (f x) -> p f x", f=2)[:, :, qi * qw : (qi + 1) * qw],
                        in_=nf_t[:].rearrange("(f p) x -> p f x", p=128)[
                            :, :, qi * qw : (qi + 1) * qw
                        ],
                    ).then_inc(nft_q[qi], 16)

                sync.dma_start(
                    out=wext_sb[:, :].rearrange("p (f x) -> p f x", f=2),
                    in_=w_ext[:].rearrange("(f p) x -> p f x", p=128),
                ).then_inc(pre_dma, 16)
                nft_quarter(0)
                issued = 1
                for b in range(nblk):
                    if issued < 4:
                        nft_quarter(issued)
                        issued += 1
                    if b >= nbuf:
                        # superblock of block b-nbuf accumulated => slot free
                        sync.wait_ge(dve_add, nr * ((b - nbuf) // SB2 + 1))
                    sync.dma_start(
                        out=att[b % nbuf][:, :],
                        in_=a_h[b * JC : (b + 1) * JC, :],
                    ).then_inc(in_dma_s[b % nbuf], 16)
                while issued < 4:
                    nft_quarter(issued)
                    issued += 1
                for r in range(nr):
                    sync.wait_ge(act_out, r + 1)
                    sync.dma_start(
                        out=out[r * RT : (r + 1) * RT, :],
                        in_=res[:, r * D : (r + 1) * D],
                    ).then_inc(out_dma, 16)
                sync.wait_ge(out_dma, 16 * nr)

            @block.vector
            def _(vector):
                nc.vector.memset(negones[0:1, :], -1.0).then_inc(dve_sem, 1)
                nc.vector.memset(zeros[:, :], 0.0).then_inc(dve_sem, 1)
                # s_dst extraction: strided [128, 2] copy per drained pair
                for q in range(npair):
                    vector.wait_ge(pe_pre, 4 * (q + 1))
                    pv = pband[q % 2][:, :].rearrange("p (t e) -> p t e", e=DEXT)
                    nc.vector.tensor_copy(sdst[:, 2 * q : 2 * q + 2], pv[:, :, D]).then_inc(dve_pre, 1)
                vector.wait_ge(dve_pre, npair)  # own writes of sdst
                nc.vector.reduce_max(out=mcol[:, :], in_=sdst[:, :], axis=AX.X).then_inc(dve_sem, 1)
                vector.wait_ge(pe_misc, 1)
                nc.vector.reduce_max(out=gmax[0:1, :], in_=misc_ps[0:1, :], axis=AX.X).then_inc(dve_sem, 1)
                vector.wait_ge(act_misc, 1)
                nc.vector.tensor_scalar_add(gall[:, :], sdst[:, :], negm[:, 0:1]).then_inc(dve_sem, 1)
                vector.wait_ge(dve_sem, 5)  # engine pipelines; adjacent RAW needs a sem
                nc.vector.tensor_scalar_max(gall[:, :], gall[:, :], -80.0).then_inc(dve_sem, 1)
                # main-loop accumulation: psum group G -> racc[r].  On the
                # final superblock, chase each add with that r's epilogue
                # (Z clamp + reciprocal) so relu/DMA overlap the remaining adds.
                for B in range(nsb):
                    for r in range(nr):
                        G = B * nr + r
                        vector.wait_ge(pe_grp, cs * (G + 1))
                        dst = racc[:, r * DEXT : (r + 1) * DEXT]
                        src = outp[G % nps][:, :]
                        if B == 0:
                            nc.vector.tensor_copy(dst, src).then_inc(dve_add, 1)
                        else:
                            vector.wait_ge(dve_add, (B - 1) * nr + r + 1)
                            nc.vector.tensor_add(dst, dst, src).then_inc(dve_add, 1)
                        if B == nsb - 1:
                            vector.wait_ge(dve_add, B * nr + r + 1)  # own add
                            # clamp Z away from 0 so padded rows (Z=0) stay
                            # finite; real rows have Z >= exp(-80) >> 1e-37
                            nc.vector.tensor_scalar_max(
                                zcl[:, r : r + 1],
                                racc[:, r * DEXT + D : (r + 1) * DEXT],
                                1e-37,
                            ).then_inc(dve_sem, 1)
                            vector.wait_ge(dve_sem, 6 + 2 * r + 1)
                            nc.vector.reciprocal(
                                rec[:, r : r + 1], zcl[:, r : r + 1]
                            ).then_inc(dve_sem, 1)

            @block.scalar
            def _(scalar):
                # drain h' (both chunks of a pair in one op; bf16 cast).  The
                # pair's s_dst columns land in hg as garbage bf16 and are
                # overwritten by g below.
                for q in range(npair):
                    scalar.wait_ge(pe_pre, 4 * (q + 1))
                    nc.scalar.copy(
                        hg_sb[:, 2 * q * DEXT : (2 * q + 2) * DEXT],
                        pband[q % 2][:, :],
                    ).then_inc(act_hp, 1)
                scalar.wait_ge(pe_misc, 2)
                nc.scalar.copy(negm[:, 0:1], misc_ps[:, 0:1]).then_inc(act_misc, 1)
                scalar.wait_ge(dve_sem, 6)
                nc.scalar.activation(
                    gall[:, :], gall[:, :], AF.Exp, bias=zeros[:, 0:1]
                ).then_inc(act_misc, 1)
                scalar.wait_ge(act_misc, 2)  # exp -> g-col copy is same-engine RAW
                scalar.wait_ge(act_hp, npair)  # muls read own earlier hg copies
                nc.scalar.copy(hg_gcol, gall[:, :]).then_inc(act_hg, 1)
                for k in range(nj):
                    nc.scalar.mul(
                        hg_sb[:, k * DEXT : k * DEXT + D],
                        hg_sb[:, k * DEXT : k * DEXT + D],
                        gall[:, k : k + 1],
                    ).then_inc(act_hg, 1)
                for r in range(nr):
                    scalar.wait_ge(dve_sem, 6 + 2 * r + 2)
                    nc.scalar.activation(
                        res[:, r * D : (r + 1) * D],
                        racc[:, r * DEXT : r * DEXT + D],
                        AF.Relu,
                        bias=zeros[:, 0:1],
                        scale=rec[:, r : r + 1],
                    ).then_inc(act_out, 1)

            @block.tensor
            def _(tensor):
                # HAM warmup: the PE clock-gates to 1.2 GHz after ~3.4us idle.
                # Fill the startup DMA wait with junk matmuls so preprocessing
                # runs at 2.4 GHz.  Nothing reads misc_ps before the (much
                # later, same-engine-ordered) transpose overwrites it.
                tensor.wait_ge(dve_sem, 1)  # negones memset done
                for _ in range(60):
                    nc.tensor.matmul(
                        misc_ps[:, :], lhsT=negones[0:1, :], rhs=negones[0:1, :],
                        start=True, stop=True,
                    )
                tensor.wait_ge(pre_dma, 16)  # wext
                qw = np_ // 4
                seen_q = -1
                for q in range(npair):
                    need_q = ((2 * q + 2) * JC - 1) // qw
                    if need_q > seen_q:
                        tensor.wait_ge(nft_q[need_q], 16)
                        seen_q = need_q
                    if q >= 2:
                        tensor.wait_ge(act_hp, q - 1)
                        tensor.wait_ge(dve_pre, q - 1)
                    for h in range(2):
                        k = 2 * q + h
                        for f in range(2):
                            nc.tensor.matmul(
                                pband[q % 2][:, h * DEXT : (h + 1) * DEXT],
                                lhsT=nft_sb[:, f * np_ + k * JC : f * np_ + (k + 1) * JC],
                                rhs=wext_sb[:, f * DEXT : (f + 1) * DEXT],
                                start=(f == 0),
                                stop=(f == 1),
                            ).then_inc(pe_pre, 1)
                tensor.wait_ge(gp_sem, 2)
                tensor.wait_ge(dve_sem, 3)
                nc.tensor.transpose(
                    misc_ps[0:1, :], mcol[:, 0:1], ident[:, :]
                ).then_inc(pe_misc, 1)
                tensor.wait_ge(dve_sem, 4)
                nc.tensor.matmul(
                    misc_ps[:, 0:1],
                    lhsT=negones[0:1, :],
                    rhs=gmax[0:1, 0:1],
                    start=True,
                    stop=True,
                ).then_inc(pe_misc, 1)
                # keep the PE warm across the exp/hg-finalize wait before the
                # main loop (group 0 clears outp[0] with start=True anyway)
                for _ in range(40):
                    nc.tensor.matmul(
                        outp[0][:, :D], lhsT=negones[0:1, :], rhs=negones[0:1, :],
                        start=True, stop=True,
                    )
                for B in range(nsb):
                    for s in range(SB2):
                        b = B * SB2 + s
                        tensor.wait_ge(in_dma_s[b % nbuf], 16 * (b // nbuf + 1))
                    tensor.wait_ge(act_hg, cs * (B + 1) + 1)
                    for r in range(nr):
                        G = B * nr + r
                        if G >= nps:
                            # psum slot free once its previous group is accumulated
                            tensor.wait_ge(dve_add, G - nps + 1)
                        for cc in range(cs):
                            b = B * SB2 + cc // C4
                            c = cc % C4
                            k = C4 * b + c
                            nc.tensor.matmul(
                                outp[G % nps][:, :],
                                lhsT=att[b % nbuf][:, c * ip + r * RT : c * ip + (r + 1) * RT],
                                rhs=hg_sb[:, k * DEXT : (k + 1) * DEXT],
                                start=(cc == 0),
                                stop=(cc == cs - 1),
                            ).then_inc(pe_grp, 1)

    return nc


_program_cache = {}


def _get_program():
    if "nc" not in _program_cache:
        _program_cache["nc"] = _build_program()
    return _program_cache["nc"]


def _prep_inputs(node_feats, Ahat, w, w_a, a, n=N, slab=SLAB, np_=NP, ip=IP, ncores=NCORES):
    node_feats = np.asarray(node_feats, dtype=np.float32)
    w = np.asarray(w, dtype=np.float32)
    w_a = np.asarray(w_a, dtype=np.float32)
    a = np.asarray(a, dtype=np.float32).reshape(4)
    Ahat = np.asarray(Ahat)

    u = w @ (w_a @ a[2:])                            # [FIN] folded s_dst weight
    w_ext = np.concatenate([w, u[:, None]], axis=1)  # [FIN, DEXT]
    nf_t = np.zeros((FIN, np_), dtype=np.float32)    # zero-pad: inert fake nodes
    nf_t[:, :n] = node_feats.T

    nblk = np_ // (C4 * JC)
    in_maps = []
    for c in range(ncores):
        slab_bf = Ahat[c * slab : (c + 1) * slab, :].astype(np.float32).astype(
            mybir.dt.np(FP8)
        )
        at = np.zeros((np_, ip), dtype=mybir.dt.np(FP8))
        at[:n, :slab] = slab_bf.T
        # block-chunk: each partition's per-block data contiguous (10KB runs)
        a_h = np.ascontiguousarray(
            at.reshape(nblk, C4, JC, ip).transpose(0, 2, 1, 3).reshape(nblk * JC, C4 * ip)
        )
        in_maps.append({"a_h": a_h, "nf_t": nf_t, "w_ext": w_ext})
    return in_maps


def kernel(node_feats, Ahat, w, w_a, a, _trace=False, _trace_cores=None):
    nc = _get_program()
    in_maps = _prep_inputs(node_feats, Ahat, w, w_a, a)
    results = run_bass_kernel_spmd(
        nc,
        in_maps,
        core_ids=list(range(NCORES)),
        trace=_trace,
        trace_cores=_trace_cores,
    )
    out = np.concatenate(
        [results.results[c]["out"][:SLAB] for c in range(NCORES)], axis=0
    )
    if _trace:
        kernel.last_results = results
    return out


# revision 49
# speedup vs baseline: 1.2042x; 1.2042x over previous
"""GAT layer kernel for 8 Trainium2 NeuronCores (raw bass, explicit semaphores).

Math: the reference computes
    h_prime = node_feats @ w                      [N, D]
    s_src   = h_prime @ w_a @ a[:2]               [N]
    s_dst   = h_prime @ w_a @ a[2:]               [N]
    e[i,j]  = s_src[i] + s_dst[j], masked by Ahat, row-softmax, @ h_prime, relu.

Because e[i,j] separates into row + column terms, s_src[i] cancels in the
row-wise softmax.  With g[j] = exp(s_dst[j] - max_j s_dst[j]):
    attn[i,j] = Ahat[i,j] * g[j] / sum_j' Ahat[i,j'] * g[j']
    out       = relu( (Ahat @ [g*h_prime | g])[:, :D] / (Ahat @ g) )
i.e. the whole layer collapses to one big [N,N] x [N,D+1] matmul against the
adjacency matrix plus tiny preprocessing.  That matmul is the memory-bound
part (Ahat is 400 MB); everything else is noise.

Sharding: output rows are sharded across the 8 cores (slab of N/8 rows each).
The PE contracts along the partition dim, so each core receives its Ahat
row-slab pre-transposed (j on partitions) and cast to bf16 (exact for 0/1
adjacency values, and halves HBM traffic).  Both dims are zero-padded
(padding is inert: zero adjacency contributes nothing) so that
  - every stationary matmul tile is exactly 128 columns (enables FWL), and
  - the host pre-chunks the slab so each partition reads one contiguous
    10 KB run per block DMA (full descriptor rate).
h_prime/s_dst preprocessing runs redundantly on every core in fp32 (s_dst
feeds exp so it needs full precision; node_feats/w are tiny).

Raw bass (not Tile): this container's walrus rejects instructions carrying
more than ~2 sync waits (and >1 on self-loading fp32 matmuls), which Tile's
auto-generated kernel-tail drain always violates.  With explicit semaphores
every wait is its own instruction, which walrus accepts.
"""

import sys
from contextlib import ExitStack

sys.path.insert(0, "/opt/trn_rl_repo")

import ml_dtypes
import numpy as np

import concourse.bass as bass
from concourse import mybir
from concourse.bass_utils import run_bass_kernel_spmd

N = 10000
FIN = 256
D = 128
DEXT = D + 1          # h_prime columns + the g column (for the softmax denom)
NCORES = 8
SLAB = N // NCORES    # real output rows per core
JC = 128              # j-chunk (contraction tile)
RT = 128              # output row tile
C4 = 4                # j-chunks per A-tile DMA block
SB2 = 2               # DMA blocks per psum accumulation group
NP = 10240            # N padded to C4*JC*20
IP = 1280             # slab padded to RT*10
NBUF = 8              # A-tile buffer depth

F32 = mybir.dt.float32
BF16 = mybir.dt.bfloat16
FP8 = mybir.dt.float8e4
F32R = mybir.dt.float32r
AF = mybir.ActivationFunctionType
AX = mybir.AxisListType


def _build_program(np_=NP, ip=IP, nbuf=NBUF):
    nj = np_ // JC            # j-chunks
    npair = nj // 2           # preprocessing chunk pairs
    nr = ip // RT             # output row tiles per core
    nblk = nj // C4           # A-tile DMA blocks
    nsb = nblk // SB2         # psum accumulation superblocks
    cs = C4 * SB2             # chunks per accumulation group
    nps = 4                   # rotating psum banks for output accumulation
    bw = C4 * ip              # SBUF columns per A-tile block
    assert np_ % (SB2 * C4 * JC) == 0 and ip % RT == 0

    nc = bass.Bass("TRN2", target_bir_lowering=False, debug=False, num_devices=NCORES)

    # a_h: block-chunked slab layout [nblk*128, C4*ip]; row b*128+p holds, for
    # each chunk c of block b, the (padded) A^T row j = (b*C4+c)*128 + p.
    a_h = nc.declare_dram_parameter("a_h", [nblk * JC, bw], FP8, isOutput=False)
    nf_t = nc.declare_dram_parameter("nf_t", [FIN, np_], F32, isOutput=False)
    w_ext = nc.declare_dram_parameter("w_ext", [FIN, DEXT], F32, isOutput=False)
    out = nc.declare_dram_parameter("out", [ip, D], F32, isOutput=True)

    ctx = ExitStack()
    with ctx:
        sb = lambda name, shape, dt: ctx.enter_context(nc.sbuf_tensor(name, shape, dt))
        ps = lambda name, shape: ctx.enter_context(nc.psum_tensor(name, shape, F32))
        sem = lambda name: ctx.enter_context(nc.semaphore(name))

        nft_sb = sb("nft_sb", [128, 2 * np_], F32)
        wext_sb = sb("wext_sb", [128, 2 * DEXT], F32)
        hg_sb = sb("hg_sb", [128, nj * DEXT], BF16)
        sdst = sb("sdst", [128, nj], F32)
        gall = sb("gall", [128, nj], F32)
        mcol = sb("mcol", [128, 1], F32)
        gmax = sb("gmax", [128, 1], F32)
        negm = sb("negm", [128, 1], F32)
        negones = sb("negones", [128, 128], F32)
        zeros = sb("zeros", [128, 1], F32)
        ident = sb("ident", [128, 128], F32)
        rec = sb("rec", [128, nr], F32)
        zcl = sb("zcl", [128, nr], F32)
        res = sb("res", [128, nr * D], F32)
        racc = sb("racc", [128, nr * DEXT], F32)
        att = [sb(f"att{i}", [128, bw], FP8) for i in range(nbuf)]

        pband = [ps("pband0", [128, 2 * DEXT]), ps("pband1", [128, 2 * DEXT])]
        misc_ps = ps("misc_ps", [128, 128])
        outp = [ps(f"outp{p}", [128, DEXT]) for p in range(nps)]

        # strided column-views (one column per chunk / per pair)
        hg_gcol = hg_sb[:, :].rearrange("p (k e) -> p k e", e=DEXT)[:, :, D]

        pre_dma = sem("pre_dma")
        nft_q = [sem(f"nft_q{i}") for i in range(4)]
        # One semaphore per A-tile buffer slot: a shared counter would be
        # unsound (16 SDMA engines inc independently, so value 16 can be a mix
        # of two DMAs).  Slot reuse is serialized by the dve_add WAR gate.
        in_dma_s = [sem(f"in_dma{i}") for i in range(nbuf)]
        out_dma = sem("out_dma")
        pe_pre = sem("pe_pre")
        act_hp = sem("act_hp")    # one inc per preproc pair drained (ACT)
        dve_pre = sem("dve_pre")  # one inc per preproc pair s_dst extracted
        pe_misc = sem("pe_misc")
        act_misc = sem("act_misc")
        dve_sem = sem("dve_sem")
        act_hg = sem("act_hg")
        pe_grp = sem("pe_grp")    # one inc per output matmul
        dve_add = sem("dve_add")  # one inc per psum -> racc accumulate
        act_out = sem("act_out")
        gp_sem = sem("gp_sem")

        with nc.Block() as block:

            @block.gpsimd
            def _(gpsimd):
                # GpSimd is 8 Q7 cores; same-engine ops race without a sem.
                nc.gpsimd.memset(ident[:, :], 0.0).then_inc(gp_sem, 1)
                gpsimd.wait_ge(gp_sem, 1)
                nc.gpsimd.affine_select(
                    out=ident[:, :],
                    in_=ident[:, :],
                    compare_op=mybir.AluOpType.not_equal,
                    fill=1.0,
                    base=0,
                    pattern=[[-1, 128]],
                    channel_multiplier=1,
                ).then_inc(gp_sem, 1)

            @block.sync
            def _(sync):
                qw = np_ // 4  # nft quarter width (j columns)

                def nft_quarter(qi):
                    # both f-halves of j-quarter qi in one DMA
                    sync.dma_start(
                        out=nft_sb[:, :]
                        .rearrange("p (f x) -> p f x", f=2)[:, :, qi * qw : (qi + 1) * qw],
                        in_=nf_t[:].rearrange("(f p) x -> p f x", p=128)[
                            :, :, qi * qw : (qi + 1) * qw
                        ],
                    ).then_inc(nft_q[qi], 16)

                sync.dma_start(
                    out=wext_sb[:, :].rearrange("p (f x) -> p f x", f=2),
                    in_=w_ext[:].rearrange("(f p) x -> p f x", p=128),
                ).then_inc(pre_dma, 16)
                nft_quarter(0)
                issued = 1
                for b in range(nblk):
                    if issued < 4:
                        nft_quarter(issued)
                        issued += 1
                    if b >= nbuf:
                        # superblock of block b-nbuf accumulated => slot free
                        sync.wait_ge(dve_add, nr * ((b - nbuf) // SB2 + 1))
                    sync.dma_start(
                        out=att[b % nbuf][:, :],
                        in_=a_h[b * JC : (b + 1) * JC, :],
                    ).then_inc(in_dma_s[b % nbuf], 16)
                while issued < 4:
                    nft_quarter(issued)
                    issued += 1
                for r in range(nr):
                    sync.wait_ge(act_out, r + 1)
                    sync.dma_start(
                        out=out[r * RT : (r + 1) * RT, :],
                        in_=res[:, r * D : (r + 1) * D],
                    ).then_inc(out_dma, 16)
                sync.wait_ge(out_dma, 16 * nr)

            @block.vector
            def _(vector):
                nc.vector.memset(negones[0:1, :], -1.0).then_inc(dve_sem, 1)
                nc.vector.memset(zeros[:, :], 0.0).then_inc(dve_sem, 1)
                # s_dst extraction: strided [128, 2] copy per drained pair
                for q in range(npair):
                    vector.wait_ge(pe_pre, 4 * (q + 1))
                    pv = pband[q % 2][:, :].rearrange("p (t e) -> p t e", e=DEXT)
                    nc.vector.tensor_copy(sdst[:, 2 * q : 2 * q + 2], pv[:, :, D]).then_inc(dve_pre, 1)
                vector.wait_ge(dve_pre, npair)  # own writes of sdst
                nc.vector.reduce_max(out=mcol[:, :], in_=sdst[:, :], axis=AX.X).then_inc(dve_sem, 1)
                vector.wait_ge(pe_misc, 1)
                nc.vector.reduce_max(out=gmax[0:1, :], in_=misc_ps[0:1, :], axis=AX.X).then_inc(dve_sem, 1)
                vector.wait_ge(act_misc, 1)
                nc.vector.tensor_scalar_add(gall[:, :], sdst[:, :], negm[:, 0:1]).then_inc(dve_sem, 1)
                vector.wait_ge(dve_sem, 5)  # engine pipelines; adjacent RAW needs a sem
                nc.vector.tensor_scalar_max(gall[:, :], gall[:, :], -80.0).then_inc(dve_sem, 1)
                # main-loop accumulation: psum group G -> racc[r].  On the
                # final superblock, chase each add with that r's epilogue
                # (Z clamp + reciprocal) so relu/DMA overlap the remaining adds.
                for B in range(nsb):
                    for r in range(nr):
                        G = B * nr + r
                        vector.wait_ge(pe_grp, cs * (G + 1))
                        dst = racc[:, r * DEXT : (r + 1) * DEXT]
                        src = outp[G % nps][:, :]
                        if B == 0:
                            nc.vector.tensor_copy(dst, src).then_inc(dve_add, 1)
                        else:
                            vector.wait_ge(dve_add, (B - 1) * nr + r + 1)
                            nc.vector.tensor_add(dst, dst, src).then_inc(dve_add, 1)
                        if B == nsb - 1:
                            vector.wait_ge(dve_add, B * nr + r + 1)  # own add
                            # clamp Z away from 0 so padded rows (Z=0) stay
                            # finite; real rows have Z >= exp(-80) >> 1e-37
                            nc.vector.tensor_scalar_max(
                                zcl[:, r : r + 1],
                                racc[:, r * DEXT + D : (r + 1) * DEXT],
                                1e-37,
                            ).then_inc(dve_sem, 1)
                            vector.wait_ge(dve_sem, 6 + 2 * r + 1)
                            nc.vector.reciprocal(
                                rec[:, r : r + 1], zcl[:, r : r + 1]
                            ).then_inc(dve_sem, 1)

            @block.scalar
            def _(scalar):
                # drain h' (both chunks of a pair in one op; bf16 cast).  The
                # pair's s_dst columns land in hg as garbage bf16 and are
                # overwritten by g below.
                for q in range(npair):
                    scalar.wait_ge(pe_pre, 4 * (q + 1))
                    nc.scalar.copy(
                        hg_sb[:, 2 * q * DEXT : (2 * q + 2) * DEXT],
                        pband[q % 2][:, :],
                    ).then_inc(act_hp, 1)
                scalar.wait_ge(pe_misc, 2)
                nc.scalar.copy(negm[:, 0:1], misc_ps[:, 0:1]).then_inc(act_misc, 1)
                scalar.wait_ge(dve_sem, 6)
                nc.scalar.activation(
                    gall[:, :], gall[:, :], AF.Exp, bias=zeros[:, 0:1]
                ).then_inc(act_misc, 1)
                scalar.wait_ge(act_misc, 2)  # exp -> g-col copy is same-engine RAW
                scalar.wait_ge(act_hp, npair)  # muls read own earlier hg copies
                nc.scalar.copy(hg_gcol, gall[:, :]).then_inc(act_hg, 1)
                for k in range(nj):
                    nc.scalar.mul(
                        hg_sb[:, k * DEXT : k * DEXT + D],
                        hg_sb[:, k * DEXT : k * DEXT + D],
                        gall[:, k : k + 1],
                    ).then_inc(act_hg, 1)
                for r in range(nr):
                    scalar.wait_ge(dve_sem, 6 + 2 * r + 2)
                    nc.scalar.activation(
                        res[:, r * D : (r + 1) * D],
                        racc[:, r * DEXT : r * DEXT + D],
                        AF.Relu,
                        bias=zeros[:, 0:1],
                        scale=rec[:, r : r + 1],
                    ).then_inc(act_out, 1)

            @block.tensor
            def _(tensor):
                tensor.wait_ge(pre_dma, 16)  # wext
                qw = np_ // 4
                seen_q = -1
                for q in range(npair):
                    need_q = ((2 * q + 2) * JC - 1) // qw
                    if need_q > seen_q:
                        tensor.wait_ge(nft_q[need_q], 16)
                        seen_q = need_q
                    if q >= 2:
                        tensor.wait_ge(act_hp, q - 1)
                        tensor.wait_ge(dve_pre, q - 1)
                    for h in range(2):
                        k = 2 * q + h
                        for f in range(2):
                            nc.tensor.matmul(
                                pband[q % 2][:, h * DEXT : (h + 1) * DEXT],
                                lhsT=nft_sb[:, f * np_ + k * JC : f * np_ + (k + 1) * JC],
                                rhs=wext_sb[:, f * DEXT : (f + 1) * DEXT],
                                start=(f == 0),
                                stop=(f == 1),
                            ).then_inc(pe_pre, 1)
                tensor.wait_ge(gp_sem, 2)
                tensor.wait_ge(dve_sem, 3)
                nc.tensor.transpose(
                    misc_ps[0:1, :], mcol[:, 0:1], ident[:, :]
                ).then_inc(pe_misc, 1)
                tensor.wait_ge(dve_sem, 4)
                nc.tensor.matmul(
                    misc_ps[:, 0:1],
                    lhsT=negones[0:1, :],
                    rhs=gmax[0:1, 0:1],
                    start=True,
                    stop=True,
                ).then_inc(pe_misc, 1)
                for B in range(nsb):
                    for s in range(SB2):
                        b = B * SB2 + s
                        tensor.wait_ge(in_dma_s[b % nbuf], 16 * (b // nbuf + 1))
                    tensor.wait_ge(act_hg, cs * (B + 1) + 1)
                    for r in range(nr):
                        G = B * nr + r
                        if G >= nps:
                            # psum slot free once its previous group is accumulated
                            tensor.wait_ge(dve_add, G - nps + 1)
                        for cc in range(cs):
                            b = B * SB2 + cc // C4
                            c = cc % C4
                            k = C4 * b + c
                            nc.tensor.matmul(
                                outp[G % nps][:, :],
                                lhsT=att[b % nbuf][:, c * ip + r * RT : c * ip + (r + 1) * RT],
                                rhs=hg_sb[:, k * DEXT : (k + 1) * DEXT],
                                start=(cc == 0),
                                stop=(cc == cs - 1),
                            ).then_inc(pe_grp, 1)

    return nc


_program_cache = {}


def _get_program():
    if "nc" not in _program_cache:
        _program_cache["nc"] = _build_program()
    return _program_cache["nc"]


def _prep_inputs(node_feats, Ahat, w, w_a, a, n=N, slab=SLAB, np_=NP, ip=IP, ncores=NCORES):
    node_feats = np.asarray(node_feats, dtype=np.float32)
    w = np.asarray(w, dtype=np.float32)
    w_a = np.asarray(w_a, dtype=np.float32)
    a = np.asarray(a, dtype=np.float32).reshape(4)
    Ahat = np.asarray(Ahat)

    u = w @ (w_a @ a[2:])                            # [FIN] folded s_dst weight
    w_ext = np.concatenate([w, u[:, None]], axis=1)  # [FIN, DEXT]
    nf_t = np.zeros((FIN, np_), dtype=np.float32)    # zero-pad: inert fake nodes
    nf_t[:, :n] = node_feats.T

    nblk = np_ // (C4 * JC)
    in_maps = []
    for c in range(ncores):
        slab_bf = Ahat[c * slab : (c + 1) * slab, :].astype(np.float32).astype(
            mybir.dt.np(FP8)
        )
        at = np.zeros((np_, ip), dtype=mybir.dt.np(FP8))
        at[:n, :slab] = slab_bf.T
        # block-chunk: each partition's per-block data contiguous (10KB runs)
        a_h = np.ascontiguousarray(
            at.reshape(nblk, C4, JC, ip).transpose(0, 2, 1, 3).reshape(nblk * JC, C4 * ip)
        )
        in_maps.append({"a_h": a_h, "nf_t": nf_t, "w_ext": w_ext})
    return in_maps


def kernel(node_feats, Ahat, w, w_a, a, _trace=False, _trace_cores=None):
    nc = _get_program()
    in_maps = _prep_inputs(node_feats, Ahat, w, w_a, a)
    results = run_bass_kernel_spmd(
        nc,
        in_maps,
        core_ids=list(range(NCORES)),
        trace=_trace,
        trace_cores=_trace_cores,
    )
    out = np.concatenate(
        [results.results[c]["out"][:SLAB] for c in range(NCORES)], axis=0
    )
    if _trace:
        kernel.last_results = results
    return out


# revision 50
# speedup vs baseline: 1.2175x; 1.0110x over previous
"""GAT layer kernel for 8 Trainium2 NeuronCores (raw bass, explicit semaphores).

Math: the reference computes
    h_prime = node_feats @ w                      [N, D]
    s_src   = h_prime @ w_a @ a[:2]               [N]
    s_dst   = h_prime @ w_a @ a[2:]               [N]
    e[i,j]  = s_src[i] + s_dst[j], masked by Ahat, row-softmax, @ h_prime, relu.

Because e[i,j] separates into row + column terms, s_src[i] cancels in the
row-wise softmax.  With g[j] = exp(s_dst[j] - max_j s_dst[j]):
    attn[i,j] = Ahat[i,j] * g[j] / sum_j' Ahat[i,j'] * g[j']
    out       = relu( (Ahat @ [g*h_prime | g])[:, :D] / (Ahat @ g) )
i.e. the whole layer collapses to one big [N,N] x [N,D+1] matmul against the
adjacency matrix plus tiny preprocessing.  That matmul is the memory-bound
part (Ahat is 400 MB); everything else is noise.

Sharding: output rows are sharded across the 8 cores (slab of N/8 rows each).
The PE contracts along the partition dim, so each core receives its Ahat
row-slab pre-transposed (j on partitions) and cast to bf16 (exact for 0/1
adjacency values, and halves HBM traffic).  Both dims are zero-padded
(padding is inert: zero adjacency contributes nothing) so that
  - every stationary matmul tile is exactly 128 columns (enables FWL), and
  - the host pre-chunks the slab so each partition reads one contiguous
    10 KB run per block DMA (full descriptor rate).
h_prime/s_dst preprocessing runs redundantly on every core in fp32 (s_dst
feeds exp so it needs full precision; node_feats/w are tiny).

Raw bass (not Tile): this container's walrus rejects instructions carrying
more than ~2 sync waits (and >1 on self-loading fp32 matmuls), which Tile's
auto-generated kernel-tail drain always violates.  With explicit semaphores
every wait is its own instruction, which walrus accepts.
"""

import sys
from contextlib import ExitStack

sys.path.insert(0, "/opt/trn_rl_repo")

import ml_dtypes
import numpy as np

import concourse.bass as bass
from concourse import mybir
from concourse.bass_utils import run_bass_kernel_spmd

N = 10000
FIN = 256
D = 128
DEXT = D + 1          # h_prime columns + the g column (for the softmax denom)
NCORES = 8
SLAB = N // NCORES    # real output rows per core
JC = 128              # j-chunk (contraction tile)
RT = 128              # output row tile
C4 = 4                # j-chunks per A-tile DMA block
SB2 = 4               # DMA blocks per psum accumulation group
NP = 10240            # N padded to C4*JC*20
IP = 1280             # slab padded to RT*10
NBUF = 10             # A-tile buffer depth

F32 = mybir.dt.float32
BF16 = mybir.dt.bfloat16
FP8 = mybir.dt.float8e4
F32R = mybir.dt.float32r
AF = mybir.ActivationFunctionType
AX = mybir.AxisListType


def _build_program(np_=NP, ip=IP, nbuf=NBUF):
    nj = np_ // JC            # j-chunks
    npair = nj // 2           # preprocessing chunk pairs
    nr = ip // RT             # output row tiles per core
    nblk = nj // C4           # A-tile DMA blocks
    nsb = nblk // SB2         # psum accumulation superblocks
    cs = C4 * SB2             # chunks per accumulation group
    nps = 4                   # rotating psum banks for output accumulation
    bw = C4 * ip              # SBUF columns per A-tile block
    assert np_ % (SB2 * C4 * JC) == 0 and ip % RT == 0

    nc = bass.Bass("TRN2", target_bir_lowering=False, debug=False, num_devices=NCORES)

    # a_h: block-chunked slab layout [nblk*128, C4*ip]; row b*128+p holds, for
    # each chunk c of block b, the (padded) A^T row j = (b*C4+c)*128 + p.
    a_h = nc.declare_dram_parameter("a_h", [nblk * JC, bw], FP8, isOutput=False)
    nf_t = nc.declare_dram_parameter("nf_t", [FIN, np_], F32, isOutput=False)
    w_ext = nc.declare_dram_parameter("w_ext", [FIN, DEXT], F32, isOutput=False)
    out = nc.declare_dram_parameter("out", [ip, D], F32, isOutput=True)

    ctx = ExitStack()
    with ctx:
        sb = lambda name, shape, dt: ctx.enter_context(nc.sbuf_tensor(name, shape, dt))
        ps = lambda name, shape: ctx.enter_context(nc.psum_tensor(name, shape, F32))
        sem = lambda name: ctx.enter_context(nc.semaphore(name))

        nft_sb = sb("nft_sb", [128, 2 * np_], F32)
        wext_sb = sb("wext_sb", [128, 2 * DEXT], F32)
        hg_sb = sb("hg_sb", [128, nj * DEXT], BF16)
        sdst = sb("sdst", [128, nj], F32)
        gall = sb("gall", [128, nj], F32)
        mcol = sb("mcol", [128, 1], F32)
        gmax = sb("gmax", [128, 1], F32)
        negm = sb("negm", [128, 1], F32)
        negones = sb("negones", [128, 128], F32)
        zeros = sb("zeros", [128, 1], F32)
        ident = sb("ident", [128, 128], F32)
        rec = sb("rec", [128, nr], F32)
        zcl = sb("zcl", [128, nr], F32)
        res = sb("res", [128, nr * D], F32)
        racc = sb("racc", [128, nr * DEXT], F32)
        att = [sb(f"att{i}", [128, bw], FP8) for i in range(nbuf)]

        pband = [ps("pband0", [128, 2 * DEXT]), ps("pband1", [128, 2 * DEXT])]
        misc_ps = ps("misc_ps", [128, 128])
        outp = [ps(f"outp{p}", [128, DEXT]) for p in range(nps)]

        # strided column-views (one column per chunk / per pair)
        hg_gcol = hg_sb[:, :].rearrange("p (k e) -> p k e", e=DEXT)[:, :, D]

        pre_dma = sem("pre_dma")
        nft_q = [sem(f"nft_q{i}") for i in range(4)]
        # One semaphore per A-tile buffer slot: a shared counter would be
        # unsound (16 SDMA engines inc independently, so value 16 can be a mix
        # of two DMAs).  Slot reuse is serialized by the dve_add WAR gate.
        in_dma_s = [sem(f"in_dma{i}") for i in range(nbuf)]
        out_dma = sem("out_dma")
        pe_pre = sem("pe_pre")
        act_hp = sem("act_hp")    # one inc per preproc pair drained (ACT)
        dve_pre = sem("dve_pre")  # one inc per preproc pair s_dst extracted
        pe_misc = sem("pe_misc")
        act_misc = sem("act_misc")
        dve_sem = sem("dve_sem")
        act_hg = sem("act_hg")
        pe_grp = sem("pe_grp")    # one inc per output matmul
        dve_add = sem("dve_add")  # one inc per psum -> racc accumulate
        act_out = sem("act_out")
        gp_sem = sem("gp_sem")

        with nc.Block() as block:

            @block.gpsimd
            def _(gpsimd):
                # GpSimd is 8 Q7 cores; same-engine ops race without a sem.
                nc.gpsimd.memset(ident[:, :], 0.0).then_inc(gp_sem, 1)
                gpsimd.wait_ge(gp_sem, 1)
                nc.gpsimd.affine_select(
                    out=ident[:, :],
                    in_=ident[:, :],
                    compare_op=mybir.AluOpType.not_equal,
                    fill=1.0,
                    base=0,
                    pattern=[[-1, 128]],
                    channel_multiplier=1,
                ).then_inc(gp_sem, 1)

            @block.sync
            def _(sync):
                qw = np_ // 4  # nft quarter width (j columns)

                def nft_quarter(qi):
                    # both f-halves of j-quarter qi in one DMA
                    sync.dma_start(
                        out=nft_sb[:, :]
                        .rearrange("p (f x) -> p f x", f=2)[:, :, qi * qw : (qi + 1) * qw],
                        in_=nf_t[:].rearrange("(f p) x -> p f x", p=128)[
                            :, :, qi * qw : (qi + 1) * qw
                        ],
                    ).then_inc(nft_q[qi], 16)

                sync.dma_start(
                    out=wext_sb[:, :].rearrange("p (f x) -> p f x", f=2),
                    in_=w_ext[:].rearrange("(f p) x -> p f x", p=128),
                ).then_inc(pre_dma, 16)
                nft_quarter(0)
                issued = 1
                for b in range(nblk):
                    if issued < 4:
                        nft_quarter(issued)
                        issued += 1
                    if b >= nbuf:
                        # superblock of block b-nbuf accumulated => slot free
                        sync.wait_ge(dve_add, nr * ((b - nbuf) // SB2 + 1))
                    sync.dma_start(
                        out=att[b % nbuf][:, :],
                        in_=a_h[b * JC : (b + 1) * JC, :],
                    ).then_inc(in_dma_s[b % nbuf], 16)
                while issued < 4:
                    nft_quarter(issued)
                    issued += 1
                for r in range(nr):
                    sync.wait_ge(act_out, r + 1)
                    sync.dma_start(
                        out=out[r * RT : (r + 1) * RT, :],
                        in_=res[:, r * D : (r + 1) * D],
                    ).then_inc(out_dma, 16)
                sync.wait_ge(out_dma, 16 * nr)

            @block.vector
            def _(vector):
                nc.vector.memset(negones[0:1, :], -1.0).then_inc(dve_sem, 1)
                nc.vector.memset(zeros[:, :], 0.0).then_inc(dve_sem, 1)
                # s_dst extraction: strided [128, 2] copy per drained pair
                for q in range(npair):
                    vector.wait_ge(pe_pre, 4 * (q + 1))
                    pv = pband[q % 2][:, :].rearrange("p (t e) -> p t e", e=DEXT)
                    nc.vector.tensor_copy(sdst[:, 2 * q : 2 * q + 2], pv[:, :, D]).then_inc(dve_pre, 1)
                vector.wait_ge(dve_pre, npair)  # own writes of sdst
                nc.vector.reduce_max(out=mcol[:, :], in_=sdst[:, :], axis=AX.X).then_inc(dve_sem, 1)
                vector.wait_ge(pe_misc, 1)
                nc.vector.reduce_max(out=gmax[0:1, :], in_=misc_ps[0:1, :], axis=AX.X).then_inc(dve_sem, 1)
                vector.wait_ge(act_misc, 1)
                nc.vector.tensor_scalar_add(gall[:, :], sdst[:, :], negm[:, 0:1]).then_inc(dve_sem, 1)
                vector.wait_ge(dve_sem, 5)  # engine pipelines; adjacent RAW needs a sem
                nc.vector.tensor_scalar_max(gall[:, :], gall[:, :], -80.0).then_inc(dve_sem, 1)
                # main-loop accumulation: psum group G -> racc[r].  On the
                # final superblock, chase each add with that r's epilogue
                # (Z clamp + reciprocal) so relu/DMA overlap the remaining adds.
                for B in range(nsb):
                    for r in range(nr):
                        G = B * nr + r
                        vector.wait_ge(pe_grp, cs * (G + 1))
                        dst = racc[:, r * DEXT : (r + 1) * DEXT]
                        src = outp[G % nps][:, :]
                        if B == 0:
                            nc.vector.tensor_copy(dst, src).then_inc(dve_add, 1)
                        else:
                            vector.wait_ge(dve_add, (B - 1) * nr + r + 1)
                            nc.vector.tensor_add(dst, dst, src).then_inc(dve_add, 1)
                        if B == nsb - 1:
                            vector.wait_ge(dve_add, B * nr + r + 1)  # own add
                            # clamp Z away from 0 so padded rows (Z=0) stay
                            # finite; real rows have Z >= exp(-80) >> 1e-37
                            nc.vector.tensor_scalar_max(
                                zcl[:, r : r + 1],
                                racc[:, r * DEXT + D : (r + 1) * DEXT],
                                1e-37,
                            ).then_inc(dve_sem, 1)
                            vector.wait_ge(dve_sem, 6 + 2 * r + 1)
                            nc.vector.reciprocal(
                                rec[:, r : r + 1], zcl[:, r : r + 1]
                            ).then_inc(dve_sem, 1)

            @block.scalar
            def _(scalar):
                # drain h' (both chunks of a pair in one op; bf16 cast).  The
                # pair's s_dst columns land in hg as garbage bf16 and are
                # overwritten by g below.
                for q in range(npair):
                    scalar.wait_ge(pe_pre, 4 * (q + 1))
                    nc.scalar.copy(
                        hg_sb[:, 2 * q * DEXT : (2 * q + 2) * DEXT],
                        pband[q % 2][:, :],
                    ).then_inc(act_hp, 1)
                scalar.wait_ge(pe_misc, 2)
                nc.scalar.copy(negm[:, 0:1], misc_ps[:, 0:1]).then_inc(act_misc, 1)
                scalar.wait_ge(dve_sem, 6)
                nc.scalar.activation(
                    gall[:, :], gall[:, :], AF.Exp, bias=zeros[:, 0:1]
                ).then_inc(act_misc, 1)
                scalar.wait_ge(act_misc, 2)  # exp -> g-col copy is same-engine RAW
                scalar.wait_ge(act_hp, npair)  # muls read own earlier hg copies
                nc.scalar.copy(hg_gcol, gall[:, :]).then_inc(act_hg, 1)
                for k in range(nj):
                    nc.scalar.mul(
                        hg_sb[:, k * DEXT : k * DEXT + D],
                        hg_sb[:, k * DEXT : k * DEXT + D],
                        gall[:, k : k + 1],
                    ).then_inc(act_hg, 1)
                for r in range(nr):
                    scalar.wait_ge(dve_sem, 6 + 2 * r + 2)
                    nc.scalar.activation(
                        res[:, r * D : (r + 1) * D],
                        racc[:, r * DEXT : r * DEXT + D],
                        AF.Relu,
                        bias=zeros[:, 0:1],
                        scale=rec[:, r : r + 1],
                    ).then_inc(act_out, 1)

            @block.tensor
            def _(tensor):
                tensor.wait_ge(pre_dma, 16)  # wext
                qw = np_ // 4
                seen_q = -1
                for q in range(npair):
                    need_q = ((2 * q + 2) * JC - 1) // qw
                    if need_q > seen_q:
                        tensor.wait_ge(nft_q[need_q], 16)
                        seen_q = need_q
                    if q >= 2:
                        tensor.wait_ge(act_hp, q - 1)
                        tensor.wait_ge(dve_pre, q - 1)
                    for h in range(2):
                        k = 2 * q + h
                        for f in range(2):
                            nc.tensor.matmul(
                                pband[q % 2][:, h * DEXT : (h + 1) * DEXT],
                                lhsT=nft_sb[:, f * np_ + k * JC : f * np_ + (k + 1) * JC],
                                rhs=wext_sb[:, f * DEXT : (f + 1) * DEXT],
                                start=(f == 0),
                                stop=(f == 1),
                            ).then_inc(pe_pre, 1)
                tensor.wait_ge(gp_sem, 2)
                tensor.wait_ge(dve_sem, 3)
                nc.tensor.transpose(
                    misc_ps[0:1, :], mcol[:, 0:1], ident[:, :]
                ).then_inc(pe_misc, 1)
                tensor.wait_ge(dve_sem, 4)
                nc.tensor.matmul(
                    misc_ps[:, 0:1],
                    lhsT=negones[0:1, :],
                    rhs=gmax[0:1, 0:1],
                    start=True,
                    stop=True,
                ).then_inc(pe_misc, 1)
                for B in range(nsb):
                    for s in range(SB2):
                        b = B * SB2 + s
                        tensor.wait_ge(in_dma_s[b % nbuf], 16 * (b // nbuf + 1))
                    tensor.wait_ge(act_hg, cs * (B + 1) + 1)
                    for r in range(nr):
                        G = B * nr + r
                        if G >= nps:
                            # psum slot free once its previous group is accumulated
                            tensor.wait_ge(dve_add, G - nps + 1)
                        for cc in range(cs):
                            b = B * SB2 + cc // C4
                            c = cc % C4
                            k = C4 * b + c
                            nc.tensor.matmul(
                                outp[G % nps][:, :],
                                lhsT=att[b % nbuf][:, c * ip + r * RT : c * ip + (r + 1) * RT],
                                rhs=hg_sb[:, k * DEXT : (k + 1) * DEXT],
                                start=(cc == 0),
                                stop=(cc == cs - 1),
                            ).then_inc(pe_grp, 1)

    return nc


_program_cache = {}


def _get_program():
    if "nc" not in _program_cache:
        _program_cache["nc"] = _build_program()
    return _program_cache["nc"]


def _prep_inputs(node_feats, Ahat, w, w_a, a, n=N, slab=SLAB, np_=NP, ip=IP, ncores=NCORES):
    node_feats = np.asarray(node_feats, dtype=np.float32)
    w = np.asarray(w, dtype=np.float32)
    w_a = np.asarray(w_a, dtype=np.float32)
    a = np.asarray(a, dtype=np.float32).reshape(4)
    Ahat = np.asarray(Ahat)

    u = w @ (w_a @ a[2:])                            # [FIN] folded s_dst weight
    w_ext = np.concatenate([w, u[:, None]], axis=1)  # [FIN, DEXT]
    nf_t = np.zeros((FIN, np_), dtype=np.float32)    # zero-pad: inert fake nodes
    nf_t[:, :n] = node_feats.T

    nblk = np_ // (C4 * JC)
    in_maps = []
    for c in range(ncores):
        slab_bf = Ahat[c * slab : (c + 1) * slab, :].astype(np.float32).astype(
            mybir.dt.np(FP8)
        )
        at = np.zeros((np_, ip), dtype=mybir.dt.np(FP8))
        at[:n, :slab] = slab_bf.T
        # block-chunk: each partition's per-block data contiguous (10KB runs)
        a_h = np.ascontiguousarray(
            at.reshape(nblk, C4, JC, ip).transpose(0, 2, 1, 3).reshape(nblk * JC, C4 * ip)
        )
        in_maps.append({"a_h": a_h, "nf_t": nf_t, "w_ext": w_ext})
    return in_maps


def kernel(node_feats, Ahat, w, w_a, a, _trace=False, _trace_cores=None):
    nc = _get_program()
    in_maps = _prep_inputs(node_feats, Ahat, w, w_a, a)
    results = run_bass_kernel_spmd(
        nc,
        in_maps,
        core_ids=list(range(NCORES)),
        trace=_trace,
        trace_cores=_trace_cores,
    )
    out = np.concatenate(
        [results.results[c]["out"][:SLAB] for c in range(NCORES)], axis=0
    )
    if _trace:
        kernel.last_results = results
    return out
